# revision 39
# baseline (speedup 1.0000x reference)
"""Trainium2 Bass kernel for nn_ADVI (segment_reduce ELBO).

Math:
  elbo = const(prior - q) + sum_n LSE_c( ll[n,c] + log_pis[ks_n, c, ts_n] )
  log_pis[k,c,t] = b_c + beta[c,t]*y[k,t] - L[k,t]   (L = LSE_c of the first part)
  The -L[k,t] term is class-independent -> sum_n L[ks_n,ts_n] is computed on host.
  Remaining device math per spike:  A[n,c] = s~^T Pt_c s~ + g_n * beta[c, t_n]
  with s~ = [s;1], g_n = y[ks_n, ts_n], and Pt_c carrying b_c + all constants in
  its (10,10) entry.  The quadratic is fit EXACTLY (res ~2e-6) as
  sum_m lam[m,c] (v_m . s~)^2 over 62 shared directions; two extra exact
  "directions" ((g+1)/2)^2 and ((g-1)/2)^2 with coefficients +-beta[c,t]
  reconstruct g*beta.  Spikes are host-sorted into 128 t-buckets so each
  128-pair matmul window uses one lam_t; the window->t map is static and
  identical on all 8 cores (per-bucket window counts are globally padded).

  Device pipeline per chunk (4096 spikes = 2048 pair-columns, 2 spikes/col),
  tuned for a PE pinned at 1.2 GHz (HAM never unthrottles here, so the PE
  column-stream 2048 (mm1) + 1024 (mm2) cols/chunk is the hard floor and
  every other engine is budgeted under it):
  DMA sp -> PE mm1 (4x512 cols into one 4-bank U) -> square: ACT Square on
  U[:,0:1024], DVE cast+tensor_tensor on U[:,1024:2048] (both emit f16 phi)
  -> PE mm2 phi_w^T @ lam_t (16 windows, llv [128 pairs, 64] = A|B classes;
  walrus --enable-ldw-opt hides the per-window LDWEIGHTS under the matmul
  stream) -> ACT exp (bf16) -> segmented 32-class reduce split: DVE does
  tree stage 1 at 2x rate, gpsimd (otherwise idle, SBUF-only) does stages
  2-5 into contrib fp32.  contrib is DMA'd home in five pieces as reduces
  land; the final ln + sum run on host in f64 (saves the second ACT table
  load + tail).  The last two chunks reduce fully on DVE (idle at the tail,
  skips the gpsimd hop).  Startup: spt chunks 0-2 are the first DMAs on
  sync (the 512KB lam pieces follow so they cannot delay spt1 in the DMA
  queue), w rides the scalar queue, and chunk 0 squares entirely on ACT to
  shorten the pipeline fill.  Because DMA completions can reorder, lam
  piece waits use a one-piece margin (p+1 pieces done implies the first p
  landed).  Semaphore count is minimized (12; stage1/gpsimd share s_red
  via doubled increments) because the bass kernel postamble costs ~0.9us
  per semaphore.
"""

import base64
import sys
from contextlib import ExitStack

import numpy as np

sys.path.insert(0, "/opt/trn_rl_repo")

import ml_dtypes  # noqa: E402

F16 = np.float16

LOG2PI = float(np.log(2.0 * np.pi))
N_K, N_T, N_C, N_D, N_SPK = 256, 128, 32, 10, 1000000
N_CORES = 8
CHUNK = 4096                      # spikes per chunk
WIN = 128                         # pairs per mm2 window (256 spikes)
M_DIRS = 62                       # fitted quadratic directions per spike

SQUARE_MODE = "castmult"          # "ttdirect" (illegal: 2 PSUM reads) | "castmult"
REDUCE_MODE = "split_tree"        # DVE stage1 + gpsimd stages 2-5
REDUCE_ENGINE = "vector"          # "gpsimd" (illegal: no free-axis reduce) | "vector"
WARMUP_MM = 0                     # PE is p-state pinned at 1.2 GHz; warm-up useless
LDW_OPT = True                    # flip walrus --enable-ldw-opt
SQ_SPLIT = 1024                   # cols squared on ACT; rest cast+mult on DVE

# 64 fitted directions (f64, 64x11) from the original reference fit; used as
# warm start for the 62-dir constrained refinement.
_V64_B64 = """
AAAAoH/q7z8AAADAZxKMPwAAAGC3gpK/AAAAYMKUkD8AAABAftuSvwAAAMD4rJq/AAAAIPyonj8AAABgqPmTPwAAAEAhMZI/AAAA
wOuImD8AAADgfAYIvwAAAAA0dIq/AAAAAG1L8D8AAABgiy+LvwAAAOChXpI/AAAAIJoyVj8AAACAp5SfPwAAACBKWni/AAAAQP9B
Qj8AAACA7E9svwAAAEAbVoO/AAAAAMFbk78AAABgyQqWPwAAAACRppK/AAAAgNFA7z8AAADgRMSUvwAAACAXYqE/AAAA4C9omL8A
AABgzk+MvwAAAACFkaE/AAAAgBYBkD8AAABAwQmbPwAAAMDHDY4/AAAAIAfUeL8AAACgFpiZvwAAAMB6ezK/AAAAoFOO8D8AAAAA
fSGRPwAAAMDzCpo/AAAAgFvYob8AAACAdd9zvwAAAAAHsYy/AAAAYDYjdj8AAAAgq6ybvwAAAGCAyZi/AAAAwORZeL8AAAAgkaqb
PwAAAGBoKJc/AAAAADpY7z8AAAAgFQabPwAAAGCR5Za/AAAAoN5dcT8AAADA7yyiPwAAACBJGWm/AAAAgMEakD8AAAAg5MGHPwAA
ACDI6om/AAAA4Bu6mr8AAAAghwSgvwAAAEBQuJ2/AAAAAIlT8D8AAACAoh2NvwAAAMDATKe/AAAAIPH4lj8AAACAN3JRPwAAAEA+
ipG/AAAA4MOIoj8AAADAWhqLvwAAAOAoZpA/AAAAIBbdkz8AAAAAv2GXvwAAAECqzJu/AAAAgDgG8D8AAAAAlYlaPwAAAGD6cIA/
AAAAgEdPmD8AAACAFWt9vwAAAGC7wZM/AAAAIG9KlL8AAABAXB2gPwAAACCKw5Q/AAAAwNc1hb8AAADAuhebPwAAAKAdFYg/AAAA
oImF8D8AAAAgVhyRvwAAAKAOMUU/AAAAoJFBq78AAADgFmd5PwAAAACdT5Q/AAAAwBoOlL8AAADAOPyOvwAAAOAhHpo/AAAAQJLF
k78AAADgQe+VPwAAAOBWfJq/AAAAgLgi8D8AAADA2yyCPwAAAOC96Y2/AAAAoCRGob8AAACASjBtPwAAACD7GIY/AAAAoLxHdj8A
AADA9/mDPwAAAGAT/YQ/AAAAQKxiZr8AAABAolWSvwAAAECaAnu/AAAAAJx/8D8AAABAzH2cvwAAACBK9+8/AAAAYAyajb8AAAAA
X8SUPwAAAGAMKZC/AAAAoF+YlD8AAADANBCYPwAAACCmSZK/AAAAwNllj78AAAAgHyeMvwAAAGBB1Zq/AAAAwHEo8D8AAACAiuyO
PwAAAGD4fO8/AAAAwJRTiz8AAADA1XWLvwAAAICtNHG/AAAA4Nbxn78AAADAWD1/PwAAAECEtD8/AAAAQPTfRT8AAABg852GPwAA
AIDISPA/AAAAwBRfkb8AAAAgOxJzPwAAAID6zO4/AAAAoJ/pfD8AAACA2zJ+vwAAAADpEoQ/AAAAAJQ7oj8AAAAAToeTvwAAAMDi
bYi/AAAAINCVj78AAACgrofwPwAAAMBaQ4w/AAAAoDJLnj8AAABAIxN8PwAAAOBBo/A/AAAAYCNFib8AAACgHnycvwAAAEDB5Kc/
AAAAIJG/jD8AAADgvKuBPwAAAABjioW/AAAAIId67j8AAABA91GXPwAAAEBYZU4/AAAAQHxIn78AAABgvBOWvwAAAIABS/A/AAAA
gNsAmL8AAABA1syXPwAAAOBNSW6/AAAAQEUUob8AAAAggTxivwAAACBQje8/AAAAoNnyhb8AAABg1N6DPwAAAGD3bmm/AAAAYCZT
mj8AAACg1kqdPwAAACBTo+4/AAAAAE4fij8AAADAnm+gPwAAAACXX4m/AAAA4HqUir8AAACAip7wPwAAAOCFKqK/AAAAIC0Qaz8A
AACgwpSavwAAAOC8aIy/AAAAACfalj8AAACg4viTPwAAAMBAte8/AAAAwHGml78AAABg6zaNvwAAAODNapC/AAAAgGk48D8AAADg
TGqcvwAAAID7tJE/AAAAYMYOob8AAAAg7A2RvwAAAIC4fWy/AAAAgNxWib8AAABgAteRvwAAACA2X+4/AAAAAKAnoD8AAAAAwJSG
vwAAAADtFPE/AAAAAA70hb8AAABAA9CTvwAAAKAxrpQ/AAAAoPUEkD8AAADAjSKYvwAAAIBnKJU/AAAAINyOmL8AAABA3Q6gPwAA
AKAsYPA/AAAAIOqql78AAACAPYzvPwAAAKAI3Zw/AAAAAOydYb8AAADAK5yAvwAAAEAQt32/AAAAoNLdhL8AAABg8g6UvwAAAOCo
cXI/AAAAIM91iz8AAADA9oyCPwAAAAAzT+8/AAAAgFRm8D8AAAAAf3PrPwAAAOBKGeI/AAAAgIl40r8AAADAE/TgvwAAAICSDrW/
AAAAIBOK6T8AAABgUrbMPwAAAIBGiuY/AAAAAIjZyD8AAACgLbjOPwAAAMB3EvI/AAAAYAxvor8AAACgqNzuPwAAAICA9ea/AAAA
4NskwT8AAAAAddHCPwAAACANYem/AAAAQMSt4L8AAAAgwJTHPwAAAIDdOqa/AAAAIIMe1j8AAADgOZmwvwAAAIAOZeE/AAAAgJKm
wj8AAACAZgDQvwAAAEDoc7m/AAAA4PbP1b8AAADA1VfYPwAAAICHW8o/AAAAoLXd3j8AAABAYhLLPwAAAKBdIrI/AAAAwHWv878A
AABgd4HjvwAAAICvlsm/AAAAAJkizr8AAADAI+LWvwAAAKBTx9e/AAAAADWflb8AAABgFzunPwAAAMARQa4/AAAAYIPZ5z8AAACg
mI/lPwAAAAAc66q/AAAAALOHxz8AAABgJ1boPwAAAKC4iNc/AAAAQByC4T8AAABADrHlvwAAAABrcry/AAAAILZ+5b8AAADAaXDw
vwAAAEDQHve/AAAAAEMNxr8AAABgc0f7PwAAAEAQHao/AAAA4Dnj2b8AAADgnjTJvwAAAIB5gNG/AAAAANUv4T8AAADgwC+jPwAA
AIC7adM/AAAAoCda4T8AAABA/CjwPwAAAOBztNA/AAAAAOFp+z8AAADgKvjJPwAAAABcaNw/AAAAgOFDtz8AAADgFR+xvwAAAMD2
ZPC/AAAAoE8h7z8AAABgIRzqPwAAAOAFl+E/AAAA4Gp7rr8AAABgbAHrPwAAAEDb37Y/AAAAIOWg578AAAAAiDruvwAAACD4pOS/
AAAAgKwc0L8AAABgtoHhvwAAAGAxoMS/AAAA4Nh81b8AAAAgfUPhvwAAAIBxJKy/AAAAoIOc7T8AAAAgiha4PwAAAGAu+/Y/AAAA
gIUH0T8AAABA88TWvwAAAMB9V+U/AAAA4PUWu78AAACAgZC+PwAAAEBuItO/AAAAYGIUnj8AAACA3EzyPwAAAGDVDtk/AAAAYP1t
+z8AAAAA7irtPwAAAOBVd7C/AAAAQGTjyL8AAAAgUcnpPwAAAIAnvqm/AAAAIJ8vUz8AAABAgQ65vwAAACB/GHw/AAAAoBK07T8A
AAAgVaDQPwAAAID8Pvu/AAAAQItmZr8AAAAAWuvzvwAAAAAM+MK/AAAA4CjN1j8AAAAAqDHVPwAAAIB858C/AAAAQHXvyD8AAACg
eYXxvwAAACBNCrS/AAAAAFOLxT8AAABgYzSyPwAAAMAM5uU/AAAAQFuAtD8AAAAgxdbUPwAAAEDha+W/AAAAQNZyzT8AAADA7iHq
PwAAAABZG+Y/AAAAAEi/1b8AAAAgVrDYvwAAAGC8oeM/AAAAgKM3fT8AAADgjq/VPwAAAEAIYu6/AAAAoFHN0j8AAABAFK+1PwAA
AKBrLcO/AAAAgHIKlz8AAADA83ruPwAAAEBkHMc/AAAAQP+iuz8AAACgIl3lvwAAAEBIWfW/AAAAoEg24D8AAABAD5DmvwAAAICR
8Lk/AAAAADnllr8AAACgAJrSvwAAAKBcgre/AAAAgMwU6D8AAAAgkqCqPwAAAEAjKr8/AAAAQF5h1L8AAAAgcib1PwAAAMAFHuM/
AAAAQF6b5D8AAABAI+3VPwAAAIBMt74/AAAA4AdRxD8AAABAVvnmPwAAAOCJStY/AAAAoM22xz8AAACgerzlvwAAAAB3Ooq/AAAA
oFWypT8AAACAkq3DPwAAAODLEdE/AAAAAMRs0j8AAADAFMXLvwAAAEDbuu2/AAAAoONnvj8AAABAJpSOvwAAAKBz8NI/AAAAAH7g
2j8AAACAmsXhPwAAAGCb8se/AAAAYIZf3j8AAAAAWEDqPwAAAOCGC6k/AAAAwBBN0j8AAADACEDlPwAAAGBDGd+/AAAAwAfY4L8A
AACgeYLxPwAAAICzEMG/AAAAICyI5r8AAADA3rv5PwAAAGDgAtK/AAAA4LpZ4r8AAABgj7ChvwAAAOCy0ti/AAAAQGyZ5r8AAABg
QX/RPwAAAKBhTcM/AAAAQG8S478AAABACtq8PwAAAOANg+I/AAAAAO96+T8AAACArrzlPwAAAIC+5PK/AAAAQG1Azr8AAACAkbCi
PwAAAEBzXuY/AAAA4Pkdrz8AAADgRKLMvwAAAAA7T8m/AAAAQM/C178AAADgrXfJvwAAAADkWbI/AAAAwMVS4r8AAAAA3GfWvwAA
AECtbde/AAAAYDdC2T8AAAAAYo7ivwAAAODOOui/AAAA4M31xj8AAACgxD+4vwAAACBvQtW/AAAAYIa/6r8AAABgqu2RvwAAAIBl
y/I/AAAAoL8Oyb8AAABgzknfvwAAAGB0D6s/AAAAYNkwkz8AAACghJPoPwAAAEAvFtI/AAAAoEz13D8AAABAPQLbPwAAACDeLbS/
AAAA4O/p+L8AAACA6QTyvwAAAED5JcU/AAAAoP6L0z8AAABgdP9XPwAAAKARV8e/AAAAgDaj6L8AAADAeTjdvwAAAABgQNS/AAAA
AGOR4b8AAADABMjDvwAAAAAowPi/AAAAoEGt0L8AAAAgRU3LvwAAAGCA7+C/AAAAAB5H2L8AAAAg6J/evwAAAOAxENg/AAAAQIOU
vL8AAAAgmSnivwAAAEBeHrg/AAAAgOpb0r8AAADASHCsvwAAAACKJs2/AAAAgGp54j8AAAAA3b3DvwAAAMBZXEe/AAAAQFR11r8A
AACAzuKgPwAAAOCcTOc/AAAAwJVtsz8AAABg7iiqvwAAAMCVEdE/AAAAIOtXlb8AAABgNh/avwAAAEAXUOC/AAAA4FG15L8AAAAg
60eivwAAAMCEVto/AAAAYFLJlr8AAACgHCPwPwAAAKBxsOg/AAAAYKygzj8AAADAH4zGvwAAACBmbvm/AAAAADI4zz8AAABg9RXh
PwAAAKAeiNc/AAAAYP6fwT8AAABgbOfTvwAAACAkSsU/AAAAYD527r8AAABgxBLvvwAAACBxttG/AAAAgNkvxT8AAACgxQb6vwAA
AICUDNs/AAAAwIM9xz8AAADgLjXiPwAAAEDbas+/AAAAYCUQ1L8AAACghOfAvwAAAOBB07c/AAAAwNAs2D8AAADgjBqjvwAAAGDZ
kum/AAAAYP0rwb8AAACg2zXHPwAAAOD3MX2/AAAAwHcypD8AAABgbZjePwAAAEBJb78/AAAA4Ky6zj8AAABACfHaPwAAAGAOpeC/
AAAAgAnHyT8AAABA66vWvwAAAECJtLI/AAAAAJWywL8AAABA9mXzvwAAAKD9H+6/AAAAgJ0S0j8AAABA2ZvZvwAAAABlAJQ/AAAA
wKY64T8AAAAgRSvQPwAAAACo6ts/AAAAoIBK7L8AAACgUCD+vwAAAKATcrU/AAAAwB0u8L8AAAAAJCXqvwAAAIBCbt0/AAAAIKXe
4L8AAABANdC1vwAAAGBGy9g/AAAAoNoDxT8AAAAAeK7fPwAAAGAYIua/AAAAgJsM/j8AAABAOVXYvwAAAEC/6cK/AAAAQMeT5L8A
AADAzirBvwAAAGA0m6k/AAAAwDXc478AAACAd9LevwAAACB6O9g/AAAAQOey8b8AAABAsSu6PwAAAGAIPZy/AAAAYA+fuz8AAACg
wnCzvwAAACCzI+E/AAAAwG4KnL8AAACg+vnyvwAAAIBVn8s/AAAAQJP0pD8AAACAH6LhPwAAAOCREtG/AAAAgOL9xT8AAADAYi22
vwAAAADJgPI/AAAAwHsd6z8AAACAYtjhPwAAACA/P9s/AAAAoLEF+D8AAADgNBjhvwAAACCPFdq/AAAAoDMWcT8AAADgaCHjPwAA
ACAtf8G/AAAAQDKGAEAAAADge4XovwAAAEBblN+/AAAAgHQe0L8AAAAgEmDMvwAAAEAhM/O/AAAAwCKf3z8AAADAsK/cPwAAAADc
xYm/AAAAwGea4L8AAAAgsj/DPwAAAIDCbgBAAAAAAATR9L8AAACgvTnYPwAAAICb38g/AAAAYEgNyb8AAADgLpm3PwAAAODNSt2/
AAAA4Dnqy78AAABgb2rkPwAAAADV9/I/AAAAoFGq1T8AAACgs7asPwAAAIA6UKW/AAAAAIsUxD8AAADAQdLmPwAAAMDJnNG/AAAA
ICnNt78AAADABcflPwAAAGCr754/AAAAwJXC6z8AAADgH1G2vwAAAGARP/a/AAAAgA9Pvb8AAACgcTrkvwAAAEB3Y8W/AAAAgDXO
8z8AAABgajPkvwAAAKBIM+m/AAAAANPO6D8AAACAOx7APwAAAAA52r+/AAAAQGFD4b8AAADAXdzwPwAAAAAGTwDAAAAAIKQU4b8A
AABg6OTIvwAAAODaBfA/AAAAgCVb2r8AAAAAw/3hvwAAAKCqAeM/AAAAwOnRij8AAADAoCLMvwAAAKBWaea/AAAA4P+q7T8AAACA
OBUAQAAAAECR/ty/AAAAoGzW5b8AAABgYS6wvwAAAOBvJKa/AAAAwN6H0D8AAAAA56zhvwAAAMDFM9a/AAAAwBsT7r8AAAAARH/Q
vwAAAOBbjLe/AAAAwHpZkL8AAACATezjvwAAAECzEHQ/AAAAQMvO5b8AAABA7VJgvwAAAGCtpuG/AAAAIG5Byb8AAAAgLjisvwAA
AOAhb8E/AAAAIN/o4j8AAACgR33iPwAAAMDPE4i/AAAAALb/zD8AAAAgUa3UPwAAAMAJR8C/AAAAAIAZ4r8AAAAAF4u7PwAAACAa
ruI/AAAAABfXzr8AAACAzfzyvwAAAKCUeN2/AAAAgL/57L8AAABA7kv7PwAAAEB7uNi/AAAAINzb0r8AAADg637OPwAAAIAzQNs/
AAAAgD/Hp78AAABgWzXYvwAAAMDUusQ/AAAAwMHP8T8AAADA4SjcPwAAAKCMA+k/AAAAwESr/D8AAACgSeiqvwAAAMBhpt0/AAAA
IGnxzz8AAACAXmPcPwAAACD+XKC/AAAAIOwc6D8AAAAAc+rePwAAAEALMqG/AAAA4F05ub8AAADAAhPIPwAAAAAp94i/AAAAYFG5
1r8AAAAAtHSTvwAAAEBUo+G/AAAA4ErdhD8AAADg7YrkPwAAAKARbMw/AAAAoCG9vr8AAADgnoyuPwAAAEAXH7M/AAAAIM5VoD8A
AACgMD2HPw==
"""


def _decode_v():
    b = base64.b64decode("".join(_V64_B64.split()))
    return np.frombuffer(b, dtype=np.float64).reshape(64, 11).copy()


# ----------------------------------------------------------------------------
# host-side math prep (small params + O(n) packing; no O(n) flops beyond
# gathers/sorts)
# ----------------------------------------------------------------------------

def _fit_dirs(P_t):
    """Fit 62 directions V (62,11) and Lam (62,32) such that
    sum_m Lam[m,c] (v_m . x)^2 ~= x^T P_t[c] x with ALL entries constrained
    (including (10,10)).  Warm-started from the stored 64-dir fit."""
    V0 = _decode_v()
    Pf = P_t.reshape(N_C, -1)                     # (32,121)
    scale = np.linalg.norm(Pf)

    def lam_and_res(V):
        B = np.einsum('mi,mj->mij', V, V).reshape(len(V), -1)
        Lam, *_ = np.linalg.lstsq(B.T, Pf.T, rcond=None)
        R = B.T @ Lam - Pf.T                      # (121,32)
        return Lam, R, B

    Lam64, _, _ = lam_and_res(V0)
    keep = np.argsort(-np.abs(Lam64).sum(axis=1))[:M_DIRS]
    V = V0[keep].copy()

    Lam, R, B = lam_and_res(V)
    m = np.zeros_like(V); v2 = np.zeros_like(V)
    lr, b1, b2, eps = 3e-3, 0.9, 0.999, 1e-8
    best = (np.inf, V.copy(), Lam)
    for it in range(8000):
        Lam, R, B = lam_and_res(V)
        loss = float(np.sum(R * R))
        if loss < best[0]:
            best = (loss, V.copy(), Lam)
        if np.sqrt(loss) / scale < 2e-6:
            break
        Rc = (R.T).reshape(N_C, 11, 11)
        G = np.einsum('mc,cij,mj->mi', Lam, Rc + Rc.transpose(0, 2, 1), V)
        m = b1 * m + (1 - b1) * G
        v2 = b2 * v2 + (1 - b2) * G * G
        mh = m / (1 - b1 ** (it + 1)); vh = v2 / (1 - b2 ** (it + 1))
        V = V - lr * mh / (np.sqrt(vh) + eps)
    loss, V, Lam = best
    return V, Lam, np.sqrt(loss) / scale


def _host_prep(s, y, ks, ts, means, covs, b_mu, b_log_sig, beta_mu, beta_log_sig):
    f8 = np.float64
    means8, covs8 = means.astype(f8), covs.astype(f8)
    P = np.linalg.inv(covs8)
    P = 0.5 * (P + P.transpose(0, 2, 1))
    sign, logdet = np.linalg.slogdet(covs8)
    assert np.all(sign > 0)

    # P_tilde (32,11,11): s~^T Pt s~ = ll[c](s) + b_c  (constants in (10,10))
    w = np.einsum('cij,cj->ci', P, means8)
    muPmu = np.einsum('ci,cij,cj->c', means8, P, means8)
    Kc = -0.5 * muPmu - 0.5 * logdet - 0.5 * N_D * LOG2PI
    Pt = np.zeros((N_C, 11, 11))
    Pt[:, :10, :10] = -0.5 * P
    Pt[:, :10, 10] = 0.5 * w
    Pt[:, 10, :10] = 0.5 * w
    Pt1010 = Kc + b_mu.astype(f8)

    # overflow shift: device logits A <= max_c(Pt1010_c + g*beta_ct); keep
    # exp(A) well inside f16 range by subtracting S from every class const.
    ks64 = ks.astype(np.int64); ts64 = ts.astype(np.int64)
    g_all = y[ks64, ts64].astype(f8)                          # (n,)
    beta8 = beta_mu.astype(f8)                                # (32,128)
    bound = (Pt1010[None, :] + g_all[:, None] * beta8[:, ts64].T).max(axis=1)
    S = float(max(0.0, bound.max() - 80.0))
    Pt[:, 10, 10] = Pt1010 - S

    V, Lam, res = _fit_dirs(Pt)

    # per-(k,t) LSE table L (f64) and its sum over spikes (host-only term)
    y8 = y.astype(f8)
    ll_kct = b_mu.astype(f8)[None, :, None] + \
        beta_mu.astype(f8)[None, :, :] * y8[:, None, :]      # (256,32,128)
    mx = ll_kct.max(axis=1)
    L = mx + np.log(np.exp(ll_kct - mx[:, None, :]).sum(axis=1))  # (256,128)
    L_sum = float(L[ks64, ts64].sum())

    # prior - q const (f64, formulas of the reference)
    lp = -0.5 * (b_mu.astype(f8) ** 2 + LOG2PI).sum() \
         - 0.5 * (beta_mu.astype(f8) ** 2 + LOG2PI).sum()
    lq = (-0.5 * LOG2PI * b_mu.size - b_log_sig.astype(f8).sum()) + \
         (-0.5 * LOG2PI * beta_mu.size - beta_log_sig.astype(f8).sum())
    elbo_const = lp - lq

    # --- bucket spikes by t; static window->t map shared by all cores ---
    order = np.argsort(ts64, kind='stable')
    counts = np.bincount(ts64, minlength=N_T)                 # (128,)
    w_t = np.maximum(1, -(-counts // (N_CORES * 2 * WIN)))    # windows/core/t
    T_wins = np.repeat(np.arange(N_T), w_t)                   # per-core windows
    n_win = len(T_wins)
    n_win_pad = -(-n_win // 16) * 16
    C = n_win_pad // 16
    T_wins = np.concatenate([T_wins, np.zeros(n_win_pad - n_win, np.int64)])
    NLOC = C * CHUNK                                          # spikes per core

    # window start offsets per bucket (in spikes, per core)
    starts = np.concatenate([[0], np.cumsum(w_t) * 2 * WIN])  # (129,)

    s_aug = np.zeros((N_SPK, 12), dtype=np.float32)
    s_aug[:, :10] = s
    s_aug[:, 10] = 1.0
    s_aug[:, 11] = g_all

    sp_cores = []
    n_real = np.zeros(N_CORES, np.int64)
    pos = 0
    rec = [np.zeros((NLOC, 12), dtype=np.float32) for _ in range(N_CORES)]
    for t in range(N_T):
        idx_t = order[pos:pos + counts[t]]
        pos += counts[t]
        splits = np.array_split(idx_t, N_CORES)
        for i in range(N_CORES):
            k = len(splits[i])
            rec[i][starts[t]:starts[t] + k] = s_aug[splits[i]]
            n_real[i] += k
    for i in range(N_CORES):
        # pack pairs: (npair, 2, 12) -> (24, npair)
        spk = rec[i].reshape(NLOC // 2, 2, 12).transpose(1, 2, 0).reshape(24, NLOC // 2)
        sp_cores.append(spk.astype(F16))
    pads_total = int(N_CORES * NLOC - n_real.sum())

    # W stationary (24, 128) bf16: per spike 62 quad dirs + 2 g-dirs
    W = np.zeros((24, 128), dtype=np.float32)
    for h in range(2):                                        # A/B half
        r0, c0 = 12 * h, 64 * h
        W[r0:r0 + 11, c0:c0 + M_DIRS] = V.T.astype(np.float32)
        W[r0 + 10, c0 + 62] = 0.5; W[r0 + 11, c0 + 62] = 0.5   # (1+g)/2
        W[r0 + 10, c0 + 63] = -0.5; W[r0 + 11, c0 + 63] = 0.5  # (g-1)/2

    # lam table (128, 128*64) bf16: per t a (128,64) block, block-diag A|B
    lam_all = np.zeros((128, N_T * 64), dtype=np.float32)
    Lam32 = Lam.astype(np.float32)                            # (62,32)
    beta32 = beta_mu.astype(np.float32)                       # (32,128)
    for t in range(N_T):
        blk = lam_all[:, 64 * t:64 * (t + 1)]
        blk[0:M_DIRS, 0:32] = Lam32
        blk[62, 0:32] = beta32[:, t]
        blk[63, 0:32] = -beta32[:, t]
        blk[64:64 + M_DIRS, 32:64] = Lam32
        blk[126, 32:64] = beta32[:, t]
        blk[127, 32:64] = -beta32[:, t]

    return dict(W=W.astype(F16), lam_all=lam_all.astype(F16),
                sp_cores=sp_cores, T_wins=tuple(int(t) for t in T_wins),
                C=C, pads_total=pads_total, L_sum=L_sum, S=S,
                n_real=int(n_real.sum()), elbo_const=elbo_const, fit_res=res)


# ----------------------------------------------------------------------------
# device graph
# ----------------------------------------------------------------------------

_GRAPHS = {}


def _build_graph(C, T_wins):
    key = (C, T_wins, SQUARE_MODE, REDUCE_MODE, REDUCE_ENGINE, WARMUP_MM,
           SQ_SPLIT)
    if key in _GRAPHS:
        return _GRAPHS[key]

    import concourse.bacc as bacc
    import concourse.mybir as mybir

    dt = mybir.dt
    AF = mybir.ActivationFunctionType
    ALU = mybir.AluOpType
    AX = mybir.AxisListType
    PF = mybir.PoolFunctionType

    nc = bacc.Bacc("TRN2")
    stk = ExitStack()

    NPAIR = C * CHUNK // 2
    sp_d = nc.declare_dram_parameter("sp", [24, NPAIR], dt.float16, isOutput=False)
    w_d = nc.declare_dram_parameter("wmat", [24, 128], dt.float16, isOutput=False)
    lam_d = nc.declare_dram_parameter("lam", [128, N_T * 64], dt.float16,
                                      isOutput=False)
    out_d = nc.declare_dram_parameter("out", [128, C * 32], dt.float32,
                                      isOutput=True)

    sb = lambda name, shape, d: stk.enter_context(nc.sbuf_tensor(name, shape, d))
    ps = lambda name, shape: stk.enter_context(nc.psum_tensor(name, shape, dt.float32))
    sem = lambda name: stk.enter_context(nc.semaphore(name))

    PCH = CHUNK // 2                  # 2048 pairs per chunk
    spt = [sb(f"spt{i}", [24, PCH], dt.float16) for i in range(3)]
    phi = [sb(f"phi{i}", [128, PCH], dt.float16) for i in range(3)]
    E = [sb(f"E{i}", [128, 32, 32], dt.bfloat16) for i in range(2)]
    contrib = sb("contrib", [128, C * 32], dt.float32)
    w_sb = sb("w_sb", [24, 128], dt.float16)
    lam_sb = sb("lam_sb", [128, N_T * 64], dt.float16)
    if SQUARE_MODE == "castmult":
        ucp = [sb(f"ucp{i}", [128, 2048 - SQ_SPLIT], dt.float16)
               for i in range(2)]
    tr1 = [sb(f"tr1_{i}", [128, 32, 16], dt.bfloat16) for i in range(2)]
    tr2 = sb("tr2", [128, 32, 8], dt.bfloat16)
    tr3 = sb("tr3", [128, 32, 4], dt.bfloat16)
    tr4 = sb("tr4", [128, 32, 2], dt.bfloat16)

    U = ps("U", [128, 2048])
    llv = [ps(f"llv{i}", [128, 1024]) for i in range(2)]

    s_in = [sem(f"s_in{i}") for i in range(3)]
    s_ld = sem("s_ld")
    s_u = sem("s_u"); s_mm2 = sem("s_mm2")
    s_sq = sem("s_sq"); s_tt = sem("s_tt"); s_cast = sem("s_cast")
    s_exp = sem("s_exp"); s_red = sem("s_red")
    s_out = sem("s_out")

    # lam table arrives in 4 pieces; chunk cm needs buckets up to its max t
    lam_piece = [min(4, T_wins[16 * cm + 15] // 32 + 1) for cm in range(C)]

    with nc.Block() as block:

        QUARTERS = [(0, (C + 3) // 4), (1, (C + 1) // 2), (2, (3 * C + 3) // 4),
                    (3, C - 2), (4, C)]

        @block.sync
        def _(e):
            # spt0 first (largest blocker); w arrives via the vector queue.
            e.dma_start(out=spt[0][:], in_=sp_d[:, 0:PCH]).then_inc(s_in[0], 16)
            e.dma_start(out=spt[1][:], in_=sp_d[:, PCH:2 * PCH]
                        ).then_inc(s_in[1], 16)
            e.dma_start(out=spt[2][:], in_=sp_d[:, 2 * PCH:3 * PCH]
                        ).then_inc(s_in[2], 16)
            # lam piece 0 (512KB) goes after the early spt chunks: it is only
            # needed by mm2(0) (~15us) but would delay spt1 in the DMA queue.
            e.dma_start(out=lam_sb[:, 0:2048], in_=lam_d[:, 0:2048]
                        ).then_inc(s_ld, 16)
            for p in range(1, 4):
                e.dma_start(out=lam_sb[:, 2048 * p:2048 * (p + 1)],
                            in_=lam_d[:, 2048 * p:2048 * (p + 1)]
                            ).then_inc(s_ld, 16)
            for c in range(3, C):
                e.wait_ge(s_u, 2 * (c - 2))                   # spt buf reuse
                e.dma_start(out=spt[c % 3][:], in_=sp_d[:, c * PCH:(c + 1) * PCH]
                            ).then_inc(s_in[c % 3], 16)
            # ship contrib home in quarters as the reduces land
            for q, cend in QUARTERS:
                cstart = 0 if q == 0 else QUARTERS[q - 1][1]
                e.wait_ge(s_red, 2 * min(cend, C - 2) + max(0, cend - (C - 2)))
                e.dma_start(out=out_d[:, 32 * cstart:32 * cend],
                            in_=contrib[:, 32 * cstart:32 * cend]
                            ).then_inc(s_out, 16)
            e.wait_ge(s_out, 96)

        @block.tensor
        def _(e):
            e.wait_ge(s_out, 16)                              # w loaded
            lam_seen = 0
            for g in range(C + 1):
                if g < C:
                    c = g
                    e.wait_ge(s_in[c % 3], 16 * (c // 3 + 1))
                    for j in range(4):
                        if j == 0 and c >= 1:
                            e.wait_ge(s_sq, c)                # U cols 0:1024 free
                        if j == 2:
                            if c == 1:
                                e.wait_ge(s_tt, 1)            # chunk0 ACT sq-b
                            elif c >= 2:
                                e.wait_ge(s_cast, c - 1)      # U tail free
                        mm = e.matmul(U[:, j * 512:(j + 1) * 512],
                                      w_sb[:], spt[c % 3][:, j * 512:(j + 1) * 512],
                                      start=True, stop=True)
                        if j in (1, 3):
                            mm.then_inc(s_u, 1)
                cm = g - 1
                if 0 <= cm:
                    if lam_piece[cm] > lam_seen:
                        lam_seen = lam_piece[cm]
                        # DMA completions may reorder: waiting for one extra
                        # piece guarantees the first lam_seen pieces landed.
                        e.wait_ge(s_ld, min(16 * (lam_seen + 1), 64))
                    if g == C:
                        e.wait_ge(s_sq, cm + 1)
                    # phi DVE-half readiness: tt(cm) (or chunk0's ACT sq-b)
                    e.wait_ge(s_tt, cm + 1)
                    if cm >= 2:
                        e.wait_ge(s_exp, cm - 1)               # llv buf reuse
                    for wd in range(16):
                        t = T_wins[16 * cm + wd]
                        mm = e.matmul(llv[cm % 2][:, 64 * wd:64 * wd + 64],
                                      phi[cm % 3][:, 128 * wd:128 * (wd + 1)],
                                      lam_sb[:, 64 * t:64 * t + 64],
                                      start=True, stop=True)
                        if wd == 15:
                            mm.then_inc(s_mm2, 1)

        def emit_reduce(e, c2):
            e.wait_ge(s_exp, c2 + 1)
            if REDUCE_MODE == "pool":
                e.pool(contrib[:, 32 * c2:32 * c2 + 32],
                       E[c2 % 2][:], PF.avg).then_inc(s_red, 1)
            elif REDUCE_MODE == "tree":
                Ei = E[c2 % 2]
                e.tensor_tensor(tr1[:], Ei[:, :, 0:16], Ei[:, :, 16:32], ALU.add)
                e.tensor_tensor(tr2[:], tr1[:, :, 0:8], tr1[:, :, 8:16], ALU.add)
                e.tensor_tensor(tr3[:], tr2[:, :, 0:4], tr2[:, :, 4:8], ALU.add)
                e.tensor_tensor(tr4[:], tr3[:, :, 0:2], tr3[:, :, 2:4], ALU.add)
                e.tensor_tensor(contrib[:, 32 * c2:32 * c2 + 32],
                                tr4[:, :, 0:1], tr4[:, :, 1:2],
                                ALU.add).then_inc(s_red, 1)
            else:
                e.tensor_reduce(contrib[:, 32 * c2:32 * c2 + 32],
                                E[c2 % 2][:], AX.X, ALU.add).then_inc(s_red, 1)

        @block.scalar
        def _(e):
            for g in range(C + 2):
                c1 = g
                if c1 == 0:                                    # chunk 0: all-ACT
                    e.wait_ge(s_u, 1)
                    e.activation(phi[0][:, 0:SQ_SPLIT], U[:, 0:SQ_SPLIT],
                                 AF.Square).then_inc(s_sq, 1)
                    e.wait_ge(s_u, 2)
                    e.activation(phi[0][:, SQ_SPLIT:2048], U[:, SQ_SPLIT:2048],
                                 AF.Square).then_inc(s_tt, 1)
                elif c1 < C:                                   # square ACT share
                    e.wait_ge(s_u, 2 * c1 + 1)
                    if c1 >= 2:
                        # covers phi[c1%3] reuse (mm2(c1-3) done) and llv
                        # readiness for the exp below (mm2(c1-2) done)
                        e.wait_ge(s_mm2, c1 - 1)
                    e.activation(phi[c1 % 3][:, 0:SQ_SPLIT], U[:, 0:SQ_SPLIT],
                                 AF.Square).then_inc(s_sq, 1)
                c2 = g - 2
                if 0 <= c2 < C:                                # exp
                    if g >= C:
                        e.wait_ge(s_mm2, c2 + 1)
                    if c2 >= 2:                                # E buf reuse
                        e.wait_ge(s_red, 2 * (c2 - 2) + 1)
                    e.activation(E[c2 % 2][:], llv[c2 % 2][:],
                                 AF.Exp).then_inc(s_exp, 1)

        @block.vector
        def _(e):
            for g in range(C):
                c1 = g
                if 1 <= c1 < C:                                # square DVE share
                    e.wait_ge(s_u, 2 * c1 + 2)
                    e.tensor_copy(ucp[c1 % 2][:],
                                  U[:, SQ_SPLIT:2048]).then_inc(s_cast, 1)
                    if c1 >= 3:
                        e.wait_ge(s_mm2, c1 - 2)               # phi buf reuse
                    e.tensor_tensor(phi[c1 % 3][:, SQ_SPLIT:2048],
                                    ucp[c1 % 2][:], ucp[c1 % 2][:],
                                    ALU.mult).then_inc(s_tt, 1)
                c2 = g - 2
                if 0 <= c2 < C:                                # reduce stage 1
                    e.wait_ge(s_exp, c2 + 1)
                    if c2 >= 2:
                        # 2*c2-1 is unreachable without GPS(c2-2) having fired
                        e.wait_ge(s_red, 2 * c2 - 1)           # tr1 buf reuse
                    Ei = E[c2 % 2]
                    e.tensor_tensor(tr1[c2 % 2][:], Ei[:, :, 0:16],
                                    Ei[:, :, 16:32], ALU.add).then_inc(s_red, 1)
            for c2 in (C - 2, C - 1):
                # tail: full reduce on DVE (idle here) skips the gpsimd hop
                e.wait_ge(s_exp, c2 + 1)
                e.tensor_reduce(contrib[:, 32 * c2:32 * c2 + 32],
                                E[c2 % 2][:], AX.X, ALU.add).then_inc(s_red, 1)

        @block.gpsimd
        def _(e):
            # w rides s_out: its +16 is necessarily the first (out pieces are
            # gated on s_red much later), so tensor can wait s_out>=16 for w.
            # gpsimd SWDGE issue runs parallel to sync's spt0 issue.
            e.dma_start(out=w_sb[:], in_=w_d[:]).then_inc(s_out, 16)
            for c2 in range(C - 2):                            # reduce stages 2-5
                e.wait_ge(s_red, 2 * c2 + 1)
                t1 = tr1[c2 % 2]
                e.tensor_tensor(tr2[:], t1[:, :, 0:8], t1[:, :, 8:16], ALU.add)
                e.tensor_tensor(tr3[:], tr2[:, :, 0:4], tr2[:, :, 4:8], ALU.add)
                e.tensor_tensor(tr4[:], tr3[:, :, 0:2], tr3[:, :, 2:4], ALU.add)
                e.tensor_tensor(contrib[:, 32 * c2:32 * c2 + 32],
                                tr4[:, :, 0:1], tr4[:, :, 1:2],
                                ALU.add).then_inc(s_red, 1)

    nc.compile()
    _GRAPHS[key] = nc
    return nc


# ----------------------------------------------------------------------------
# entry point
# ----------------------------------------------------------------------------

LAST_RESULTS = None


def kernel(s, y, ks, ts, means, covs, b_mu, b_log_sig, beta_mu, beta_log_sig):
    import os
    global LAST_RESULTS
    s = np.asarray(s); y = np.asarray(y)
    ks = np.asarray(ks); ts = np.asarray(ts)
    means = np.asarray(means); covs = np.asarray(covs)
    b_mu = np.asarray(b_mu); b_log_sig = np.asarray(b_log_sig)
    beta_mu = np.asarray(beta_mu); beta_log_sig = np.asarray(beta_log_sig)

    prep = _host_prep(s, y, ks, ts, means, covs, b_mu,
                      b_log_sig, beta_mu, beta_log_sig)

    nc = _build_graph(prep["C"], prep["T_wins"])
    import concourse.bass_utils as bu
    from concourse.bass_utils import run_bass_kernel_spmd
    if LDW_OPT and not getattr(bu, "_ldw_opt_patched", False):
        _orig_run = bu.run_command

        def _patched_run(cmd, *a, **kw):
            if isinstance(cmd, list):
                cmd = ["--enable-ldw-opt=true" if c == "--enable-ldw-opt=false"
                       else c for c in cmd]
            return _orig_run(cmd, *a, **kw)

        bu.run_command = _patched_run
        bu._ldw_opt_patched = True

    in_maps = []
    for i in range(N_CORES):
        in_maps.append({
            "sp": np.asarray(prep["sp_cores"][i]),
            "wmat": np.asarray(prep["W"]),
            "lam": np.asarray(prep["lam_all"]),
        })

    trace = bool(os.environ.get("BASS_TRACE"))
    res = run_bass_kernel_spmd(nc, in_maps, core_ids=list(range(N_CORES)),
                               trace=trace)
    LAST_RESULTS = res

    # device returns per-(pair, window-slot) class-sums; ln + final sum on host
    partials = [float(np.log(res.results[i]["out"].astype(np.float64)).sum())
                for i in range(N_CORES)]
    ln32 = float(np.log(32.0))
    total = (sum(partials)
             + prep["S"] * prep["n_real"]
             - prep["L_sum"]
             + prep["elbo_const"]
             - prep["pads_total"] * ln32)
    return np.float32(total)


# revision 40
# speedup vs baseline: 1.1937x; 1.1937x over previous
"""Trainium2 Bass kernel for nn_ADVI (segment_reduce ELBO).

Math:
  elbo = const(prior - q) + sum_n LSE_c( ll[n,c] + log_pis[ks_n, c, ts_n] )
  log_pis[k,c,t] = b_c + beta[c,t]*y[k,t] - L[k,t]   (L = LSE_c of the first part)
  The -L[k,t] term is class-independent -> sum_n L[ks_n,ts_n] is computed on host.
  Remaining device math per spike:  A[n,c] = s~^T Pt_c s~ + g_n * beta[c, t_n]
  with s~ = [s;1], g_n = y[ks_n, ts_n], and Pt_c carrying b_c + all constants in
  its (10,10) entry.  The quadratic is fit EXACTLY (res ~2e-6) as
  sum_m lam[m,c] (v_m . s~)^2 over 62 shared directions; two extra exact
  "directions" ((g+1)/2)^2 and ((g-1)/2)^2 with coefficients +-beta[c,t]
  reconstruct g*beta.  Spikes are host-sorted into 128 t-buckets so each
  128-pair matmul window uses one lam_t; the window->t map is static and
  identical on all 8 cores (per-bucket window counts are globally padded).

  Device pipeline per chunk (4096 spikes = 2048 pair-columns, 2 spikes/col),
  tuned for a PE pinned at 1.2 GHz (HAM never unthrottles here, so the PE
  column-stream 2048 (mm1) + 1024 (mm2) cols/chunk is the hard floor and
  every other engine is budgeted under it):
  DMA sp -> PE mm1 (4x512 cols into one 4-bank U) -> square: ACT Square on
  U[:,0:1024], DVE cast+tensor_tensor on U[:,1024:2048] (both emit f16 phi)
  -> PE mm2 phi_w^T @ lam_t (16 windows, llv [128 pairs, 64] = A|B classes;
  walrus --enable-ldw-opt hides the per-window LDWEIGHTS under the matmul
  stream) -> ACT exp (bf16) -> segmented 32-class reduce split: DVE does
  tree stage 1 at 2x rate, gpsimd (otherwise idle, SBUF-only) does stages
  2-5 into contrib fp32.  contrib is DMA'd home in five pieces as reduces
  land; the final ln + sum run on host in f64 (saves the second ACT table
  load + tail).  The last two chunks reduce fully on DVE (idle at the tail,
  skips the gpsimd hop).  Startup: spt chunks 0-2 are the first DMAs on
  sync (the 512KB lam pieces follow so they cannot delay spt1 in the DMA
  queue), w rides the scalar queue, and chunk 0 squares entirely on ACT to
  shorten the pipeline fill.  Because DMA completions can reorder, lam
  piece waits use a one-piece margin (p+1 pieces done implies the first p
  landed).  Semaphore count is minimized (12; stage1/gpsimd share s_red
  via doubled increments) because the bass kernel postamble costs ~0.9us
  per semaphore.
"""

import base64
import sys
from contextlib import ExitStack

import numpy as np

sys.path.insert(0, "/opt/trn_rl_repo")

import ml_dtypes  # noqa: E402

F16 = np.float16

LOG2PI = float(np.log(2.0 * np.pi))
N_K, N_T, N_C, N_D, N_SPK = 256, 128, 32, 10, 1000000
N_CORES = 8
CHUNK = 4096                      # spikes per chunk
WIN = 128                         # pairs per mm2 window (256 spikes)
M_DIRS = 62                       # fitted quadratic directions per spike

SQUARE_MODE = "castmult"          # "ttdirect" (illegal: 2 PSUM reads) | "castmult"
REDUCE_MODE = "split_tree"        # DVE stage1 + gpsimd stages 2-5
REDUCE_ENGINE = "vector"          # "gpsimd" (illegal: no free-axis reduce) | "vector"
WARMUP_MM = 0                     # PE is p-state pinned at 1.2 GHz; warm-up useless
LDW_OPT = True                    # flip walrus --enable-ldw-opt
SQ_SPLIT = 1024                   # cols squared on ACT; rest cast+mult on DVE

# 64 fitted directions (f64, 64x11) from the original reference fit; used as
# warm start for the 62-dir constrained refinement.
_V64_B64 = """
AAAAoH/q7z8AAADAZxKMPwAAAGC3gpK/AAAAYMKUkD8AAABAftuSvwAAAMD4rJq/AAAAIPyonj8AAABgqPmTPwAAAEAhMZI/AAAA
wOuImD8AAADgfAYIvwAAAAA0dIq/AAAAAG1L8D8AAABgiy+LvwAAAOChXpI/AAAAIJoyVj8AAACAp5SfPwAAACBKWni/AAAAQP9B
Qj8AAACA7E9svwAAAEAbVoO/AAAAAMFbk78AAABgyQqWPwAAAACRppK/AAAAgNFA7z8AAADgRMSUvwAAACAXYqE/AAAA4C9omL8A
AABgzk+MvwAAAACFkaE/AAAAgBYBkD8AAABAwQmbPwAAAMDHDY4/AAAAIAfUeL8AAACgFpiZvwAAAMB6ezK/AAAAoFOO8D8AAAAA
fSGRPwAAAMDzCpo/AAAAgFvYob8AAACAdd9zvwAAAAAHsYy/AAAAYDYjdj8AAAAgq6ybvwAAAGCAyZi/AAAAwORZeL8AAAAgkaqb
PwAAAGBoKJc/AAAAADpY7z8AAAAgFQabPwAAAGCR5Za/AAAAoN5dcT8AAADA7yyiPwAAACBJGWm/AAAAgMEakD8AAAAg5MGHPwAA
ACDI6om/AAAA4Bu6mr8AAAAghwSgvwAAAEBQuJ2/AAAAAIlT8D8AAACAoh2NvwAAAMDATKe/AAAAIPH4lj8AAACAN3JRPwAAAEA+
ipG/AAAA4MOIoj8AAADAWhqLvwAAAOAoZpA/AAAAIBbdkz8AAAAAv2GXvwAAAECqzJu/AAAAgDgG8D8AAAAAlYlaPwAAAGD6cIA/
AAAAgEdPmD8AAACAFWt9vwAAAGC7wZM/AAAAIG9KlL8AAABAXB2gPwAAACCKw5Q/AAAAwNc1hb8AAADAuhebPwAAAKAdFYg/AAAA
oImF8D8AAAAgVhyRvwAAAKAOMUU/AAAAoJFBq78AAADgFmd5PwAAAACdT5Q/AAAAwBoOlL8AAADAOPyOvwAAAOAhHpo/AAAAQJLF
k78AAADgQe+VPwAAAOBWfJq/AAAAgLgi8D8AAADA2yyCPwAAAOC96Y2/AAAAoCRGob8AAACASjBtPwAAACD7GIY/AAAAoLxHdj8A
AADA9/mDPwAAAGAT/YQ/AAAAQKxiZr8AAABAolWSvwAAAECaAnu/AAAAAJx/8D8AAABAzH2cvwAAACBK9+8/AAAAYAyajb8AAAAA
X8SUPwAAAGAMKZC/AAAAoF+YlD8AAADANBCYPwAAACCmSZK/AAAAwNllj78AAAAgHyeMvwAAAGBB1Zq/AAAAwHEo8D8AAACAiuyO
PwAAAGD4fO8/AAAAwJRTiz8AAADA1XWLvwAAAICtNHG/AAAA4Nbxn78AAADAWD1/PwAAAECEtD8/AAAAQPTfRT8AAABg852GPwAA
AIDISPA/AAAAwBRfkb8AAAAgOxJzPwAAAID6zO4/AAAAoJ/pfD8AAACA2zJ+vwAAAADpEoQ/AAAAAJQ7oj8AAAAAToeTvwAAAMDi
bYi/AAAAINCVj78AAACgrofwPwAAAMBaQ4w/AAAAoDJLnj8AAABAIxN8PwAAAOBBo/A/AAAAYCNFib8AAACgHnycvwAAAEDB5Kc/
AAAAIJG/jD8AAADgvKuBPwAAAABjioW/AAAAIId67j8AAABA91GXPwAAAEBYZU4/AAAAQHxIn78AAABgvBOWvwAAAIABS/A/AAAA
gNsAmL8AAABA1syXPwAAAOBNSW6/AAAAQEUUob8AAAAggTxivwAAACBQje8/AAAAoNnyhb8AAABg1N6DPwAAAGD3bmm/AAAAYCZT
mj8AAACg1kqdPwAAACBTo+4/AAAAAE4fij8AAADAnm+gPwAAAACXX4m/AAAA4HqUir8AAACAip7wPwAAAOCFKqK/AAAAIC0Qaz8A
AACgwpSavwAAAOC8aIy/AAAAACfalj8AAACg4viTPwAAAMBAte8/AAAAwHGml78AAABg6zaNvwAAAODNapC/AAAAgGk48D8AAADg
TGqcvwAAAID7tJE/AAAAYMYOob8AAAAg7A2RvwAAAIC4fWy/AAAAgNxWib8AAABgAteRvwAAACA2X+4/AAAAAKAnoD8AAAAAwJSG
vwAAAADtFPE/AAAAAA70hb8AAABAA9CTvwAAAKAxrpQ/AAAAoPUEkD8AAADAjSKYvwAAAIBnKJU/AAAAINyOmL8AAABA3Q6gPwAA
AKAsYPA/AAAAIOqql78AAACAPYzvPwAAAKAI3Zw/AAAAAOydYb8AAADAK5yAvwAAAEAQt32/AAAAoNLdhL8AAABg8g6UvwAAAOCo
cXI/AAAAIM91iz8AAADA9oyCPwAAAAAzT+8/AAAAgFRm8D8AAAAAf3PrPwAAAOBKGeI/AAAAgIl40r8AAADAE/TgvwAAAICSDrW/
AAAAIBOK6T8AAABgUrbMPwAAAIBGiuY/AAAAAIjZyD8AAACgLbjOPwAAAMB3EvI/AAAAYAxvor8AAACgqNzuPwAAAICA9ea/AAAA
4NskwT8AAAAAddHCPwAAACANYem/AAAAQMSt4L8AAAAgwJTHPwAAAIDdOqa/AAAAIIMe1j8AAADgOZmwvwAAAIAOZeE/AAAAgJKm
wj8AAACAZgDQvwAAAEDoc7m/AAAA4PbP1b8AAADA1VfYPwAAAICHW8o/AAAAoLXd3j8AAABAYhLLPwAAAKBdIrI/AAAAwHWv878A
AABgd4HjvwAAAICvlsm/AAAAAJkizr8AAADAI+LWvwAAAKBTx9e/AAAAADWflb8AAABgFzunPwAAAMARQa4/AAAAYIPZ5z8AAACg
mI/lPwAAAAAc66q/AAAAALOHxz8AAABgJ1boPwAAAKC4iNc/AAAAQByC4T8AAABADrHlvwAAAABrcry/AAAAILZ+5b8AAADAaXDw
vwAAAEDQHve/AAAAAEMNxr8AAABgc0f7PwAAAEAQHao/AAAA4Dnj2b8AAADgnjTJvwAAAIB5gNG/AAAAANUv4T8AAADgwC+jPwAA
AIC7adM/AAAAoCda4T8AAABA/CjwPwAAAOBztNA/AAAAAOFp+z8AAADgKvjJPwAAAABcaNw/AAAAgOFDtz8AAADgFR+xvwAAAMD2
ZPC/AAAAoE8h7z8AAABgIRzqPwAAAOAFl+E/AAAA4Gp7rr8AAABgbAHrPwAAAEDb37Y/AAAAIOWg578AAAAAiDruvwAAACD4pOS/
AAAAgKwc0L8AAABgtoHhvwAAAGAxoMS/AAAA4Nh81b8AAAAgfUPhvwAAAIBxJKy/AAAAoIOc7T8AAAAgiha4PwAAAGAu+/Y/AAAA
gIUH0T8AAABA88TWvwAAAMB9V+U/AAAA4PUWu78AAACAgZC+PwAAAEBuItO/AAAAYGIUnj8AAACA3EzyPwAAAGDVDtk/AAAAYP1t
+z8AAAAA7irtPwAAAOBVd7C/AAAAQGTjyL8AAAAgUcnpPwAAAIAnvqm/AAAAIJ8vUz8AAABAgQ65vwAAACB/GHw/AAAAoBK07T8A
AAAgVaDQPwAAAID8Pvu/AAAAQItmZr8AAAAAWuvzvwAAAAAM+MK/AAAA4CjN1j8AAAAAqDHVPwAAAIB858C/AAAAQHXvyD8AAACg
eYXxvwAAACBNCrS/AAAAAFOLxT8AAABgYzSyPwAAAMAM5uU/AAAAQFuAtD8AAAAgxdbUPwAAAEDha+W/AAAAQNZyzT8AAADA7iHq
PwAAAABZG+Y/AAAAAEi/1b8AAAAgVrDYvwAAAGC8oeM/AAAAgKM3fT8AAADgjq/VPwAAAEAIYu6/AAAAoFHN0j8AAABAFK+1PwAA
AKBrLcO/AAAAgHIKlz8AAADA83ruPwAAAEBkHMc/AAAAQP+iuz8AAACgIl3lvwAAAEBIWfW/AAAAoEg24D8AAABAD5DmvwAAAICR
8Lk/AAAAADnllr8AAACgAJrSvwAAAKBcgre/AAAAgMwU6D8AAAAgkqCqPwAAAEAjKr8/AAAAQF5h1L8AAAAgcib1PwAAAMAFHuM/
AAAAQF6b5D8AAABAI+3VPwAAAIBMt74/AAAA4AdRxD8AAABAVvnmPwAAAOCJStY/AAAAoM22xz8AAACgerzlvwAAAAB3Ooq/AAAA
oFWypT8AAACAkq3DPwAAAODLEdE/AAAAAMRs0j8AAADAFMXLvwAAAEDbuu2/AAAAoONnvj8AAABAJpSOvwAAAKBz8NI/AAAAAH7g
2j8AAACAmsXhPwAAAGCb8se/AAAAYIZf3j8AAAAAWEDqPwAAAOCGC6k/AAAAwBBN0j8AAADACEDlPwAAAGBDGd+/AAAAwAfY4L8A
AACgeYLxPwAAAICzEMG/AAAAICyI5r8AAADA3rv5PwAAAGDgAtK/AAAA4LpZ4r8AAABgj7ChvwAAAOCy0ti/AAAAQGyZ5r8AAABg
QX/RPwAAAKBhTcM/AAAAQG8S478AAABACtq8PwAAAOANg+I/AAAAAO96+T8AAACArrzlPwAAAIC+5PK/AAAAQG1Azr8AAACAkbCi
PwAAAEBzXuY/AAAA4Pkdrz8AAADgRKLMvwAAAAA7T8m/AAAAQM/C178AAADgrXfJvwAAAADkWbI/AAAAwMVS4r8AAAAA3GfWvwAA
AECtbde/AAAAYDdC2T8AAAAAYo7ivwAAAODOOui/AAAA4M31xj8AAACgxD+4vwAAACBvQtW/AAAAYIa/6r8AAABgqu2RvwAAAIBl
y/I/AAAAoL8Oyb8AAABgzknfvwAAAGB0D6s/AAAAYNkwkz8AAACghJPoPwAAAEAvFtI/AAAAoEz13D8AAABAPQLbPwAAACDeLbS/
AAAA4O/p+L8AAACA6QTyvwAAAED5JcU/AAAAoP6L0z8AAABgdP9XPwAAAKARV8e/AAAAgDaj6L8AAADAeTjdvwAAAABgQNS/AAAA
AGOR4b8AAADABMjDvwAAAAAowPi/AAAAoEGt0L8AAAAgRU3LvwAAAGCA7+C/AAAAAB5H2L8AAAAg6J/evwAAAOAxENg/AAAAQIOU
vL8AAAAgmSnivwAAAEBeHrg/AAAAgOpb0r8AAADASHCsvwAAAACKJs2/AAAAgGp54j8AAAAA3b3DvwAAAMBZXEe/AAAAQFR11r8A
AACAzuKgPwAAAOCcTOc/AAAAwJVtsz8AAABg7iiqvwAAAMCVEdE/AAAAIOtXlb8AAABgNh/avwAAAEAXUOC/AAAA4FG15L8AAAAg
60eivwAAAMCEVto/AAAAYFLJlr8AAACgHCPwPwAAAKBxsOg/AAAAYKygzj8AAADAH4zGvwAAACBmbvm/AAAAADI4zz8AAABg9RXh
PwAAAKAeiNc/AAAAYP6fwT8AAABgbOfTvwAAACAkSsU/AAAAYD527r8AAABgxBLvvwAAACBxttG/AAAAgNkvxT8AAACgxQb6vwAA
AICUDNs/AAAAwIM9xz8AAADgLjXiPwAAAEDbas+/AAAAYCUQ1L8AAACghOfAvwAAAOBB07c/AAAAwNAs2D8AAADgjBqjvwAAAGDZ
kum/AAAAYP0rwb8AAACg2zXHPwAAAOD3MX2/AAAAwHcypD8AAABgbZjePwAAAEBJb78/AAAA4Ky6zj8AAABACfHaPwAAAGAOpeC/
AAAAgAnHyT8AAABA66vWvwAAAECJtLI/AAAAAJWywL8AAABA9mXzvwAAAKD9H+6/AAAAgJ0S0j8AAABA2ZvZvwAAAABlAJQ/AAAA
wKY64T8AAAAgRSvQPwAAAACo6ts/AAAAoIBK7L8AAACgUCD+vwAAAKATcrU/AAAAwB0u8L8AAAAAJCXqvwAAAIBCbt0/AAAAIKXe
4L8AAABANdC1vwAAAGBGy9g/AAAAoNoDxT8AAAAAeK7fPwAAAGAYIua/AAAAgJsM/j8AAABAOVXYvwAAAEC/6cK/AAAAQMeT5L8A
AADAzirBvwAAAGA0m6k/AAAAwDXc478AAACAd9LevwAAACB6O9g/AAAAQOey8b8AAABAsSu6PwAAAGAIPZy/AAAAYA+fuz8AAACg
wnCzvwAAACCzI+E/AAAAwG4KnL8AAACg+vnyvwAAAIBVn8s/AAAAQJP0pD8AAACAH6LhPwAAAOCREtG/AAAAgOL9xT8AAADAYi22
vwAAAADJgPI/AAAAwHsd6z8AAACAYtjhPwAAACA/P9s/AAAAoLEF+D8AAADgNBjhvwAAACCPFdq/AAAAoDMWcT8AAADgaCHjPwAA
ACAtf8G/AAAAQDKGAEAAAADge4XovwAAAEBblN+/AAAAgHQe0L8AAAAgEmDMvwAAAEAhM/O/AAAAwCKf3z8AAADAsK/cPwAAAADc
xYm/AAAAwGea4L8AAAAgsj/DPwAAAIDCbgBAAAAAAATR9L8AAACgvTnYPwAAAICb38g/AAAAYEgNyb8AAADgLpm3PwAAAODNSt2/
AAAA4Dnqy78AAABgb2rkPwAAAADV9/I/AAAAoFGq1T8AAACgs7asPwAAAIA6UKW/AAAAAIsUxD8AAADAQdLmPwAAAMDJnNG/AAAA
ICnNt78AAADABcflPwAAAGCr754/AAAAwJXC6z8AAADgH1G2vwAAAGARP/a/AAAAgA9Pvb8AAACgcTrkvwAAAEB3Y8W/AAAAgDXO
8z8AAABgajPkvwAAAKBIM+m/AAAAANPO6D8AAACAOx7APwAAAAA52r+/AAAAQGFD4b8AAADAXdzwPwAAAAAGTwDAAAAAIKQU4b8A
AABg6OTIvwAAAODaBfA/AAAAgCVb2r8AAAAAw/3hvwAAAKCqAeM/AAAAwOnRij8AAADAoCLMvwAAAKBWaea/AAAA4P+q7T8AAACA
OBUAQAAAAECR/ty/AAAAoGzW5b8AAABgYS6wvwAAAOBvJKa/AAAAwN6H0D8AAAAA56zhvwAAAMDFM9a/AAAAwBsT7r8AAAAARH/Q
vwAAAOBbjLe/AAAAwHpZkL8AAACATezjvwAAAECzEHQ/AAAAQMvO5b8AAABA7VJgvwAAAGCtpuG/AAAAIG5Byb8AAAAgLjisvwAA
AOAhb8E/AAAAIN/o4j8AAACgR33iPwAAAMDPE4i/AAAAALb/zD8AAAAgUa3UPwAAAMAJR8C/AAAAAIAZ4r8AAAAAF4u7PwAAACAa
ruI/AAAAABfXzr8AAACAzfzyvwAAAKCUeN2/AAAAgL/57L8AAABA7kv7PwAAAEB7uNi/AAAAINzb0r8AAADg637OPwAAAIAzQNs/
AAAAgD/Hp78AAABgWzXYvwAAAMDUusQ/AAAAwMHP8T8AAADA4SjcPwAAAKCMA+k/AAAAwESr/D8AAACgSeiqvwAAAMBhpt0/AAAA
IGnxzz8AAACAXmPcPwAAACD+XKC/AAAAIOwc6D8AAAAAc+rePwAAAEALMqG/AAAA4F05ub8AAADAAhPIPwAAAAAp94i/AAAAYFG5
1r8AAAAAtHSTvwAAAEBUo+G/AAAA4ErdhD8AAADg7YrkPwAAAKARbMw/AAAAoCG9vr8AAADgnoyuPwAAAEAXH7M/AAAAIM5VoD8A
AACgMD2HPw==
"""


def _decode_v():
    b = base64.b64decode("".join(_V64_B64.split()))
    return np.frombuffer(b, dtype=np.float64).reshape(64, 11).copy()


# ----------------------------------------------------------------------------
# host-side math prep (small params + O(n) packing; no O(n) flops beyond
# gathers/sorts)
# ----------------------------------------------------------------------------

def _fit_dirs(P_t):
    """Fit 62 directions V (62,11) and Lam (62,32) such that
    sum_m Lam[m,c] (v_m . x)^2 ~= x^T P_t[c] x with ALL entries constrained
    (including (10,10)).  Warm-started from the stored 64-dir fit."""
    V0 = _decode_v()
    Pf = P_t.reshape(N_C, -1)                     # (32,121)
    scale = np.linalg.norm(Pf)

    def lam_and_res(V):
        B = np.einsum('mi,mj->mij', V, V).reshape(len(V), -1)
        Lam, *_ = np.linalg.lstsq(B.T, Pf.T, rcond=None)
        R = B.T @ Lam - Pf.T                      # (121,32)
        return Lam, R, B

    Lam64, _, _ = lam_and_res(V0)
    keep = np.argsort(-np.abs(Lam64).sum(axis=1))[:M_DIRS]
    V = V0[keep].copy()

    Lam, R, B = lam_and_res(V)
    m = np.zeros_like(V); v2 = np.zeros_like(V)
    lr, b1, b2, eps = 3e-3, 0.9, 0.999, 1e-8
    best = (np.inf, V.copy(), Lam)
    for it in range(8000):
        Lam, R, B = lam_and_res(V)
        loss = float(np.sum(R * R))
        if loss < best[0]:
            best = (loss, V.copy(), Lam)
        if np.sqrt(loss) / scale < 2e-6:
            break
        Rc = (R.T).reshape(N_C, 11, 11)
        G = np.einsum('mc,cij,mj->mi', Lam, Rc + Rc.transpose(0, 2, 1), V)
        m = b1 * m + (1 - b1) * G
        v2 = b2 * v2 + (1 - b2) * G * G
        mh = m / (1 - b1 ** (it + 1)); vh = v2 / (1 - b2 ** (it + 1))
        V = V - lr * mh / (np.sqrt(vh) + eps)
    loss, V, Lam = best
    return V, Lam, np.sqrt(loss) / scale


def _host_prep(s, y, ks, ts, means, covs, b_mu, b_log_sig, beta_mu, beta_log_sig):
    f8 = np.float64
    means8, covs8 = means.astype(f8), covs.astype(f8)
    P = np.linalg.inv(covs8)
    P = 0.5 * (P + P.transpose(0, 2, 1))
    sign, logdet = np.linalg.slogdet(covs8)
    assert np.all(sign > 0)

    # P_tilde (32,11,11): s~^T Pt s~ = ll[c](s) + b_c  (constants in (10,10))
    w = np.einsum('cij,cj->ci', P, means8)
    muPmu = np.einsum('ci,cij,cj->c', means8, P, means8)
    Kc = -0.5 * muPmu - 0.5 * logdet - 0.5 * N_D * LOG2PI
    Pt = np.zeros((N_C, 11, 11))
    Pt[:, :10, :10] = -0.5 * P
    Pt[:, :10, 10] = 0.5 * w
    Pt[:, 10, :10] = 0.5 * w
    Pt1010 = Kc + b_mu.astype(f8)

    # overflow shift: device logits A <= max_c(Pt1010_c + g*beta_ct); keep
    # exp(A) well inside f16 range by subtracting S from every class const.
    ks64 = ks.astype(np.int64); ts64 = ts.astype(np.int64)
    g_all = y[ks64, ts64].astype(f8)                          # (n,)
    beta8 = beta_mu.astype(f8)                                # (32,128)
    bound = (Pt1010[None, :] + g_all[:, None] * beta8[:, ts64].T).max(axis=1)
    S = float(max(0.0, bound.max() - 80.0))
    Pt[:, 10, 10] = Pt1010 - S

    V, Lam, res = _fit_dirs(Pt)

    # per-(k,t) LSE table L (f64) and its sum over spikes (host-only term)
    y8 = y.astype(f8)
    ll_kct = b_mu.astype(f8)[None, :, None] + \
        beta_mu.astype(f8)[None, :, :] * y8[:, None, :]      # (256,32,128)
    mx = ll_kct.max(axis=1)
    L = mx + np.log(np.exp(ll_kct - mx[:, None, :]).sum(axis=1))  # (256,128)
    L_sum = float(L[ks64, ts64].sum())

    # prior - q const (f64, formulas of the reference)
    lp = -0.5 * (b_mu.astype(f8) ** 2 + LOG2PI).sum() \
         - 0.5 * (beta_mu.astype(f8) ** 2 + LOG2PI).sum()
    lq = (-0.5 * LOG2PI * b_mu.size - b_log_sig.astype(f8).sum()) + \
         (-0.5 * LOG2PI * beta_mu.size - beta_log_sig.astype(f8).sum())
    elbo_const = lp - lq

    # --- bucket spikes by t; static window->t map shared by all cores ---
    order = np.argsort(ts64, kind='stable')
    counts = np.bincount(ts64, minlength=N_T)                 # (128,)
    w_t = np.maximum(1, -(-counts // (N_CORES * 2 * WIN)))    # windows/core/t
    T_wins = np.repeat(np.arange(N_T), w_t)                   # per-core windows
    n_win = len(T_wins)
    n_win_pad = -(-n_win // 16) * 16
    C = n_win_pad // 16
    T_wins = np.concatenate([T_wins, np.zeros(n_win_pad - n_win, np.int64)])
    NLOC = C * CHUNK                                          # spikes per core

    # window start offsets per bucket (in spikes, per core)
    starts = np.concatenate([[0], np.cumsum(w_t) * 2 * WIN])  # (129,)

    s_aug = np.zeros((N_SPK, 12), dtype=np.float32)
    s_aug[:, :10] = s
    s_aug[:, 10] = 1.0
    s_aug[:, 11] = g_all

    sp_cores = []
    n_real = np.zeros(N_CORES, np.int64)
    pos = 0
    rec = [np.zeros((NLOC, 12), dtype=np.float32) for _ in range(N_CORES)]
    for t in range(N_T):
        idx_t = order[pos:pos + counts[t]]
        pos += counts[t]
        splits = np.array_split(idx_t, N_CORES)
        for i in range(N_CORES):
            k = len(splits[i])
            rec[i][starts[t]:starts[t] + k] = s_aug[splits[i]]
            n_real[i] += k
    for i in range(N_CORES):
        # pack pairs: (npair, 2, 12) -> (24, npair)
        spk = rec[i].reshape(NLOC // 2, 2, 12).transpose(1, 2, 0).reshape(24, NLOC // 2)
        sp_cores.append(spk.astype(F16))
    pads_total = int(N_CORES * NLOC - n_real.sum())

    # W stationary (24, 128) bf16: per spike 62 quad dirs + 2 g-dirs
    W = np.zeros((24, 128), dtype=np.float32)
    for h in range(2):                                        # A/B half
        r0, c0 = 12 * h, 64 * h
        W[r0:r0 + 11, c0:c0 + M_DIRS] = V.T.astype(np.float32)
        W[r0 + 10, c0 + 62] = 0.5; W[r0 + 11, c0 + 62] = 0.5   # (1+g)/2
        W[r0 + 10, c0 + 63] = -0.5; W[r0 + 11, c0 + 63] = 0.5  # (g-1)/2

    # lam table (128, 128*64) bf16: per t a (128,64) block, block-diag A|B
    lam_all = np.zeros((128, N_T * 64), dtype=np.float32)
    Lam32 = Lam.astype(np.float32)                            # (62,32)
    beta32 = beta_mu.astype(np.float32)                       # (32,128)
    for t in range(N_T):
        blk = lam_all[:, 64 * t:64 * (t + 1)]
        blk[0:M_DIRS, 0:32] = Lam32
        blk[62, 0:32] = beta32[:, t]
        blk[63, 0:32] = -beta32[:, t]
        blk[64:64 + M_DIRS, 32:64] = Lam32
        blk[126, 32:64] = beta32[:, t]
        blk[127, 32:64] = -beta32[:, t]

    return dict(W=W.astype(F16), lam_all=lam_all.astype(F16),
                sp_cores=sp_cores, T_wins=tuple(int(t) for t in T_wins),
                C=C, pads_total=pads_total, L_sum=L_sum, S=S,
                n_real=int(n_real.sum()), elbo_const=elbo_const, fit_res=res)


# ----------------------------------------------------------------------------
# device graph
# ----------------------------------------------------------------------------

_GRAPHS = {}


def _build_graph(C, T_wins):
    key = (C, T_wins, SQUARE_MODE, REDUCE_MODE, REDUCE_ENGINE, WARMUP_MM,
           SQ_SPLIT)
    if key in _GRAPHS:
        return _GRAPHS[key]

    import concourse.bacc as bacc
    import concourse.mybir as mybir

    dt = mybir.dt
    AF = mybir.ActivationFunctionType
    ALU = mybir.AluOpType
    AX = mybir.AxisListType
    PF = mybir.PoolFunctionType

    nc = bacc.Bacc("TRN2")
    stk = ExitStack()

    NPAIR = C * CHUNK // 2
    sp_d = nc.declare_dram_parameter("sp", [24, NPAIR], dt.float16, isOutput=False)
    w_d = nc.declare_dram_parameter("wmat", [24, 128], dt.float16, isOutput=False)
    lam_d = nc.declare_dram_parameter("lam", [128, N_T * 64], dt.float16,
                                      isOutput=False)
    out_d = nc.declare_dram_parameter("out", [128, C * 32], dt.float32,
                                      isOutput=True)

    sb = lambda name, shape, d: stk.enter_context(nc.sbuf_tensor(name, shape, d))
    ps = lambda name, shape: stk.enter_context(nc.psum_tensor(name, shape, dt.float32))
    sem = lambda name: stk.enter_context(nc.semaphore(name))

    PCH = CHUNK // 2                  # 2048 pairs per chunk
    spt = [sb(f"spt{i}", [24, PCH], dt.float16) for i in range(3)]
    phi = [sb(f"phi{i}", [128, PCH], dt.float16) for i in range(3)]
    E = [sb(f"E{i}", [128, 32, 32], dt.bfloat16) for i in range(2)]
    contrib = sb("contrib", [128, C * 32], dt.float32)
    w_sb = sb("w_sb", [24, 128], dt.float16)
    lam_sb = sb("lam_sb", [128, N_T * 64], dt.float16)
    if SQUARE_MODE == "castmult":
        ucp = [sb(f"ucp{i}", [128, 2048 - SQ_SPLIT], dt.float16)
               for i in range(2)]
    tr1 = [sb(f"tr1_{i}", [128, 32, 16], dt.bfloat16) for i in range(2)]
    tr2 = sb("tr2", [128, 32, 8], dt.bfloat16)
    tr3 = sb("tr3", [128, 32, 4], dt.bfloat16)
    tr4 = sb("tr4", [128, 32, 2], dt.bfloat16)

    U = ps("U", [128, 2048])
    llv = [ps(f"llv{i}", [128, 1024]) for i in range(2)]

    s_in = [sem(f"s_in{i}") for i in range(3)]
    s_ld = sem("s_ld")
    s_u = sem("s_u"); s_mm2 = sem("s_mm2")
    s_sq = sem("s_sq"); s_tt = sem("s_tt"); s_cast = sem("s_cast")
    s_exp = sem("s_exp"); s_red = sem("s_red")
    s_out = sem("s_out")

    # lam table arrives in 4 pieces; chunk cm needs buckets up to its max t
    lam_piece = [min(4, T_wins[16 * cm + 15] // 32 + 1) for cm in range(C)]

    with nc.Block() as block:

        QUARTERS = [(0, (C + 3) // 4), (1, (C + 1) // 2), (2, (3 * C + 3) // 4),
                    (3, C - 2), (4, C)]

        @block.sync
        def _(e):
            # spt0 first (largest blocker); w arrives via the vector queue.
            e.dma_start(out=spt[0][:], in_=sp_d[:, 0:PCH]).then_inc(s_in[0], 16)
            e.dma_start(out=spt[1][:], in_=sp_d[:, PCH:2 * PCH]
                        ).then_inc(s_in[1], 16)
            e.dma_start(out=spt[2][:], in_=sp_d[:, 2 * PCH:3 * PCH]
                        ).then_inc(s_in[2], 16)
            # lam piece 0 (512KB) goes after the early spt chunks: it is only
            # needed by mm2(0) (~15us) but would delay spt1 in the DMA queue.
            e.dma_start(out=lam_sb[:, 0:2048], in_=lam_d[:, 0:2048]
                        ).then_inc(s_ld, 16)
            for p in range(1, 4):
                e.dma_start(out=lam_sb[:, 2048 * p:2048 * (p + 1)],
                            in_=lam_d[:, 2048 * p:2048 * (p + 1)]
                            ).then_inc(s_ld, 16)
            for c in range(3, C):
                e.wait_ge(s_u, 2 * (c - 2))                   # spt buf reuse
                e.dma_start(out=spt[c % 3][:], in_=sp_d[:, c * PCH:(c + 1) * PCH]
                            ).then_inc(s_in[c % 3], 16)
            # ship contrib home in quarters as the reduces land
            for q, cend in QUARTERS:
                cstart = 0 if q == 0 else QUARTERS[q - 1][1]
                e.wait_ge(s_red, 2 * min(cend, C - 2) + max(0, cend - (C - 2)))
                e.dma_start(out=out_d[:, 32 * cstart:32 * cend],
                            in_=contrib[:, 32 * cstart:32 * cend]
                            ).then_inc(s_out, 16)
            e.wait_ge(s_out, 96)

        @block.tensor
        def _(e):
            e.wait_ge(s_out, 16)                              # w loaded
            lam_seen = 0
            for g in range(C + 1):
                if g < C:
                    c = g
                    e.wait_ge(s_in[c % 3], 16 * (c // 3 + 1))
                    for j in range(4):
                        if j == 0 and c >= 1:
                            e.wait_ge(s_sq, c)                # U cols 0:1024 free
                        if j == 2:
                            if c == 1:
                                e.wait_ge(s_tt, 1)            # chunk0 ACT sq-b
                            elif c >= 2:
                                e.wait_ge(s_cast, c - 1)      # U tail free
                        mm = e.matmul(U[:, j * 512:(j + 1) * 512],
                                      w_sb[:], spt[c % 3][:, j * 512:(j + 1) * 512],
                                      start=True, stop=True)
                        if j in (1, 3):
                            mm.then_inc(s_u, 1)
                cm = g - 1
                if 0 <= cm:
                    if lam_piece[cm] > lam_seen:
                        lam_seen = lam_piece[cm]
                        # DMA completions may reorder: waiting for one extra
                        # piece guarantees the first lam_seen pieces landed.
                        e.wait_ge(s_ld, min(16 * (lam_seen + 1), 64))
                    if g == C:
                        e.wait_ge(s_sq, cm + 1)
                    # phi DVE-half readiness: tt(cm) (or chunk0's ACT sq-b)
                    e.wait_ge(s_tt, cm + 1)
                    if cm >= 2:
                        e.wait_ge(s_exp, cm - 1)               # llv buf reuse
                    for wd in range(16):
                        t = T_wins[16 * cm + wd]
                        mm = e.matmul(llv[cm % 2][:, 64 * wd:64 * wd + 64],
                                      phi[cm % 3][:, 128 * wd:128 * (wd + 1)],
                                      lam_sb[:, 64 * t:64 * t + 64],
                                      start=True, stop=True)
                        if wd == 15:
                            mm.then_inc(s_mm2, 1)

        def emit_reduce(e, c2):
            e.wait_ge(s_exp, c2 + 1)
            if REDUCE_MODE == "pool":
                e.pool(contrib[:, 32 * c2:32 * c2 + 32],
                       E[c2 % 2][:], PF.avg).then_inc(s_red, 1)
            elif REDUCE_MODE == "tree":
                Ei = E[c2 % 2]
                e.tensor_tensor(tr1[:], Ei[:, :, 0:16], Ei[:, :, 16:32], ALU.add)
                e.tensor_tensor(tr2[:], tr1[:, :, 0:8], tr1[:, :, 8:16], ALU.add)
                e.tensor_tensor(tr3[:], tr2[:, :, 0:4], tr2[:, :, 4:8], ALU.add)
                e.tensor_tensor(tr4[:], tr3[:, :, 0:2], tr3[:, :, 2:4], ALU.add)
                e.tensor_tensor(contrib[:, 32 * c2:32 * c2 + 32],
                                tr4[:, :, 0:1], tr4[:, :, 1:2],
                                ALU.add).then_inc(s_red, 1)
            else:
                e.tensor_reduce(contrib[:, 32 * c2:32 * c2 + 32],
                                E[c2 % 2][:], AX.X, ALU.add).then_inc(s_red, 1)

        @block.scalar
        def _(e):
            # w rides s_out: its +16 is necessarily the first (out pieces are
            # gated on s_red much later), so tensor can wait s_out>=16 for w.
            e.dma_start(out=w_sb[:], in_=w_d[:]).then_inc(s_out, 16)
            for g in range(C + 2):
                c1 = g
                if c1 == 0:                                    # chunk 0: all-ACT
                    e.wait_ge(s_u, 1)
                    e.activation(phi[0][:, 0:SQ_SPLIT], U[:, 0:SQ_SPLIT],
                                 AF.Square).then_inc(s_sq, 1)
                    e.wait_ge(s_u, 2)
                    e.activation(phi[0][:, SQ_SPLIT:2048], U[:, SQ_SPLIT:2048],
                                 AF.Square).then_inc(s_tt, 1)
                elif c1 < C:                                   # square ACT share
                    e.wait_ge(s_u, 2 * c1 + 1)
                    if c1 >= 2:
                        # covers phi[c1%3] reuse (mm2(c1-3) done) and llv
                        # readiness for the exp below (mm2(c1-2) done)
                        e.wait_ge(s_mm2, c1 - 1)
                    e.activation(phi[c1 % 3][:, 0:SQ_SPLIT], U[:, 0:SQ_SPLIT],
                                 AF.Square).then_inc(s_sq, 1)
                c2 = g - 2
                if 0 <= c2 < C:                                # exp
                    if g >= C:
                        e.wait_ge(s_mm2, c2 + 1)
                    if c2 >= 2:                                # E buf reuse
                        e.wait_ge(s_red, 2 * (c2 - 2) + 1)
                    e.activation(E[c2 % 2][:], llv[c2 % 2][:],
                                 AF.Exp).then_inc(s_exp, 1)

        @block.vector
        def _(e):
            for g in range(C):
                c1 = g
                if 1 <= c1 < C:                                # square DVE share
                    e.wait_ge(s_u, 2 * c1 + 2)
                    e.tensor_copy(ucp[c1 % 2][:],
                                  U[:, SQ_SPLIT:2048]).then_inc(s_cast, 1)
                    if c1 >= 3:
                        e.wait_ge(s_mm2, c1 - 2)               # phi buf reuse
                    e.tensor_tensor(phi[c1 % 3][:, SQ_SPLIT:2048],
                                    ucp[c1 % 2][:], ucp[c1 % 2][:],
                                    ALU.mult).then_inc(s_tt, 1)
                c2 = g - 2
                if 0 <= c2 < C:                                # reduce stage 1
                    e.wait_ge(s_exp, c2 + 1)
                    if c2 >= 2:
                        # 2*c2-1 is unreachable without GPS(c2-2) having fired
                        e.wait_ge(s_red, 2 * c2 - 1)           # tr1 buf reuse
                    Ei = E[c2 % 2]
                    e.tensor_tensor(tr1[c2 % 2][:], Ei[:, :, 0:16],
                                    Ei[:, :, 16:32], ALU.add).then_inc(s_red, 1)
            for c2 in (C - 2, C - 1):
                # tail: full reduce on DVE (idle here) skips the gpsimd hop
                e.wait_ge(s_exp, c2 + 1)
                e.tensor_reduce(contrib[:, 32 * c2:32 * c2 + 32],
                                E[c2 % 2][:], AX.X, ALU.add).then_inc(s_red, 1)

        @block.gpsimd
        def _(e):
            for c2 in range(C - 2):                            # reduce stages 2-5
                e.wait_ge(s_red, 2 * c2 + 1)
                t1 = tr1[c2 % 2]
                e.tensor_tensor(tr2[:], t1[:, :, 0:8], t1[:, :, 8:16], ALU.add)
                e.tensor_tensor(tr3[:], tr2[:, :, 0:4], tr2[:, :, 4:8], ALU.add)
                e.tensor_tensor(tr4[:], tr3[:, :, 0:2], tr3[:, :, 2:4], ALU.add)
                e.tensor_tensor(contrib[:, 32 * c2:32 * c2 + 32],
                                tr4[:, :, 0:1], tr4[:, :, 1:2],
                                ALU.add).then_inc(s_red, 1)

    nc.compile()
    _GRAPHS[key] = nc
    return nc


# ----------------------------------------------------------------------------
# entry point
# ----------------------------------------------------------------------------

LAST_RESULTS = None


def kernel(s, y, ks, ts, means, covs, b_mu, b_log_sig, beta_mu, beta_log_sig):
    import os
    global LAST_RESULTS
    s = np.asarray(s); y = np.asarray(y)
    ks = np.asarray(ks); ts = np.asarray(ts)
    means = np.asarray(means); covs = np.asarray(covs)
    b_mu = np.asarray(b_mu); b_log_sig = np.asarray(b_log_sig)
    beta_mu = np.asarray(beta_mu); beta_log_sig = np.asarray(beta_log_sig)

    prep = _host_prep(s, y, ks, ts, means, covs, b_mu,
                      b_log_sig, beta_mu, beta_log_sig)

    nc = _build_graph(prep["C"], prep["T_wins"])
    import concourse.bass_utils as bu
    from concourse.bass_utils import run_bass_kernel_spmd
    if LDW_OPT and not getattr(bu, "_ldw_opt_patched", False):
        _orig_run = bu.run_command

        def _patched_run(cmd, *a, **kw):
            if isinstance(cmd, list):
                cmd = ["--enable-ldw-opt=true" if c == "--enable-ldw-opt=false"
                       else c for c in cmd]
            return _orig_run(cmd, *a, **kw)

        bu.run_command = _patched_run
        bu._ldw_opt_patched = True

    in_maps = []
    for i in range(N_CORES):
        in_maps.append({
            "sp": np.asarray(prep["sp_cores"][i]),
            "wmat": np.asarray(prep["W"]),
            "lam": np.asarray(prep["lam_all"]),
        })

    trace = bool(os.environ.get("BASS_TRACE"))
    res = run_bass_kernel_spmd(nc, in_maps, core_ids=list(range(N_CORES)),
                               trace=trace)
    LAST_RESULTS = res

    # device returns per-(pair, window-slot) class-sums; ln + final sum on host
    partials = [float(np.log(res.results[i]["out"].astype(np.float64)).sum())
                for i in range(N_CORES)]
    ln32 = float(np.log(32.0))
    total = (sum(partials)
             + prep["S"] * prep["n_real"]
             - prep["L_sum"]
             + prep["elbo_const"]
             - prep["pads_total"] * ln32)
    return np.float32(total)
